# revision 21
# baseline (speedup 1.0000x reference)
"""Multi-head causal self-attention (B=2, S=2048, D=1024, H=16) on 8 TRN2 cores.

Sharding: head-parallel attention + token-parallel output projection.
Core c owns head-group c = heads {2c, 2c+1} (= 128 of the 1024 qkv dims,
both batches) for stages B-E, then tokens [128c, 128c+128) of each
half-batch for stage F (Wo replicated).

Pipeline (the key scheduling idea): attention (stage D) saturates the ACT
engine (exp) while the QKV projection (stage B) and out-projection (F)
saturate only the PE. So batch 1's projection chunks are emitted INSIDE
batch 0's attention, and batch 0's out-projection inside batch 1's
attention — PE absorbs the ACT-bound windows and neither engine idles:

  B(b0) | D(b0,qc01) | B(b1,tq0) | D(b0,qc23) | B(b1,tq1)+C(b1)
        | D(b1,qc0..3) interleaved with F(b0 halves) | F(b1 halves)

Stages per core:
  B: Q^T/K^T/V^T = (x @ W{q,k,v}[:, c-slice] + b)^T  (bf16, f32 PSUM)
  C: V^T -> V_aug [tok, 65] tiles (col 64 = ones, for the l-row trick)
  D: per (batch, q-chunk, k-tile): scores^T for BOTH heads in one
     [128,1024] PSUM tile — h0 on PE rows 0-63, h1 on rows 64-127
     (different row groups -> concurrent). One exp (ACT, scale=1/8, bf16)
     covers both heads; causal mask (DVE) on diagonal tiles; ctx^T
     accumulation per head; ones column gives l = sum(exp) in row 64.
  E: per q-chunk: r = 1/l (ACT Ln+Exp), PE outer-product broadcast,
     normalize to bf16, DMA into A2A send blocks; per half-batch
     AllToAll reshards ctx^T to token-parallel (~0.25MB/rank wire).
  F: out^T[all od, my 128 tokens] = Wo^T-tiles @ resharded ctx^T + bo.

Host: passes x pre-transposed bf16, preswizzled weight slices
([128, kt*n] rows), full preswizzled Wo; reassembles per-core token
slices.
"""

import sys

for p in ("/opt/trn_rl_repo", "/root/.axon_site/_ro/trn_rl_repo"):
    if p not in sys.path:
        sys.path.insert(0, p)

import numpy as np
import ml_dtypes

import bass_rust
import concourse.bass as bass
import concourse.mybir as mybir
from concourse.bass_utils import run_bass_kernel_spmd
from concourse.masks import make_identity
from concourse.tile import TileContext

B, S, D = 2, 2048, 1024
H, DH = 16, 64
T = B * S              # 4096 tokens
NC = 8                 # cores
HG = D // NC           # 128 qkv dims per core (2 heads)
HT = 128               # tokens per rank per half-batch (stage F)
KT_D = D // 128        # 8 contraction tiles over d_model
NQC = S // 512         # 4 q-chunks per batch
INV_SCALE = 1.0 / float(np.sqrt(DH))  # 1/8
F32 = mybir.dt.float32
F32R = mybir.dt.float32r
BF16 = mybir.dt.bfloat16
BFNP = ml_dtypes.bfloat16


def _split_waits(nc, max_waits=1):
    """This walrus build accepts one sync-wait per instruction; Tile sometimes
    emits more. Split extras into preceding NoOps on the same engine."""
    n = 0
    for f in nc.m.functions:
        for bb in f.blocks:
            out = []
            for inst in bb.instructions:
                si = getattr(inst, "sync_info", None)
                if si is not None and si.on_wait and len(si.on_wait) > max_waits:
                    waits = list(si.on_wait)
                    head, rest = waits[:-max_waits], waits[-max_waits:]
                    k = 0
                    while head:
                        chunk, head = head[:max_waits], head[max_waits:]
                        out.append(mybir.InstNoOp(
                            name=f"{inst.name}-wsplit-{k}", ins=[], outs=[],
                            engine=inst.engine,
                            sync_info=bass_rust.SyncInfo(on_wait=chunk, on_update=[]),
                        ))
                        k += 1
                    si.on_wait = rest
                    n += 1
                out.append(inst)
            bb.instructions = out
    return n


def build_module(repeat=1, stages="BCDEF", do_collective=True):
    del repeat, stages  # debug knobs no longer used
    nc = bass.Bass()

    xT = nc.dram_tensor("xT", [D, T], BF16, kind="ExternalInput")
    # weights arrive host-preswizzled as [128, kt*n]: row p holds
    # W[kt*128+p, n] for kt-major n-minor — one contiguous DMA line per
    # partition instead of 1024 small descriptors
    wq = nc.dram_tensor("wq", [128, KT_D * HG], BF16, kind="ExternalInput")
    wk = nc.dram_tensor("wk", [128, KT_D * HG], BF16, kind="ExternalInput")
    wv = nc.dram_tensor("wv", [128, KT_D * HG], BF16, kind="ExternalInput")
    wo = nc.dram_tensor("wo", [128, KT_D * D], BF16, kind="ExternalInput")
    bq = nc.dram_tensor("bq", [HG, 1], F32, kind="ExternalInput")
    bk = nc.dram_tensor("bk", [HG, 1], F32, kind="ExternalInput")
    bv = nc.dram_tensor("bv", [HG, 1], F32, kind="ExternalInput")
    bo = nc.dram_tensor("bo", [128, KT_D], F32, kind="ExternalInput")
    yT = nc.dram_tensor("yT", [D, B * 2 * HT], F32, kind="ExternalOutput")

    # AllToAll staging per (batch, half): send block j = (my 128 dims,
    # 128 tokens of rank j); receive block j = (rank j's 128 dims, my
    # 128 tokens of that half)
    a2i = [[nc.dram_tensor(f"a2i{b}_{h}", [NC, HG, HT], BF16)
            for h in range(2)] for b in range(B)]
    a2o = [[nc.dram_tensor(f"a2o{b}_{h}", [NC, HG, HT], BF16)
            for h in range(2)] for b in range(B)]

    with TileContext(nc) as tc:
        with tc.tile_pool(name="persist", bufs=1) as pp:
            w_sb = {}
            for name, dram, eng in (("wq", wq, nc.sync), ("wk", wk, nc.scalar),
                                    ("wv", wv, nc.scalar)):
                t = pp.tile([128, KT_D, HG], BF16, name=f"{name}_sb", tag=f"{name}_sb")
                eng.dma_start(out=t[:], in_=dram[:].rearrange("p (kt n) -> p kt n", kt=KT_D))
                w_sb[name] = t
            # wo_sb tile allocated here; its 2MB DMA is emitted at the start
            # of stage D so it doesn't steal DMA bandwidth from the startup
            # x/w loads
            wo_sb = pp.tile([128, KT_D, D], BF16, name="wo_sb", tag="wo_sb")
            b_sb = {}
            for name, dram in (("bq", bq), ("bk", bk), ("bv", bv)):
                t = pp.tile([HG, 1], F32, name=f"{name}_sb", tag=f"{name}_sb")
                nc.sync.dma_start(out=t[:], in_=dram[:])
                b_sb[name] = t
            bo_sb = pp.tile([128, KT_D], F32, name="bo_sb", tag="bo_sb")
            nc.sync.dma_start(out=bo_sb[:], in_=bo[:])

            # identity built in f32 (gpsimd memset can't write bf16 reliably),
            # then DVE-copied (rounds) into the bf16 tile matmul needs
            ident_f = pp.tile([128, 128], F32, name="ident_f", tag="ident_f")
            make_identity(nc, ident_f[:])
            ident = pp.tile([128, 128], BF16, name="ident", tag="ident")
            nc.vector.tensor_copy(ident[:], ident_f[:])
            # multiplicative causal mask for a diagonal 128x128 tile in
            # scores^T: tri01[r, c] = 1 where r <= c (k <= q), else 0
            tri_f = pp.tile([128, 128], F32, name="tri_f", tag="tri_f")
            nc.gpsimd.memset(tri_f[:], 1.0)
            nc.gpsimd.affine_select(
                out=tri_f[:], in_=tri_f[:],
                compare_op=mybir.AluOpType.is_ge, fill=0.0,
                base=0, pattern=[[1, 128]], channel_multiplier=-1,
            )
            tri01 = pp.tile([128, 128], BF16, name="tri01", tag="tri01")
            nc.vector.tensor_copy(tri01[:], tri_f[:])
            # ones row for the r-broadcast outer product (f32r, full speed)
            ones_r = pp.tile([65, 128], F32R, name="ones_r", tag="ones_r")
            of = pp.tile([65, 128], F32, name="of", tag="of")
            nc.vector.memset(of[:], 1.0)
            nc.vector.tensor_copy(ones_r[:], of[:])
            ones128 = pp.tile([128, 64], F32, name="ones128", tag="ones128")
            nc.vector.memset(ones128[:], 1.0)

            qkvT = {}
            for name in ("qT", "kT", "vT"):
                qkvT[name] = [pp.tile([128, S], BF16, name=f"{name}{b}", tag=f"{name}{b}")
                              for b in range(B)]

            vaug = pp.tile([128, B * 2, S // 128, DH + 1], BF16, name="vaug", tag="vaug")
            nc.vector.tensor_copy(vaug[:, :, :, DH:DH + 1], ones128[:, :])
            # [65 used partitions, pair, q]; row 64 = l
            ctxu = pp.tile([128, B * 2, S], F32, name="ctxu", tag="ctxu")

            # ---------------- stage B: one (batch, 1024-token) chunk ----------------
            def stage_B(xt_pool, psB_pool, b, tq):
                t0 = tq * 1024
                xts = []
                for kt in range(KT_D):
                    xt = xt_pool.tile([128, 1024], BF16, name="xt", tag="xt")
                    nc.sync.dma_start(
                        out=xt[:],
                        in_=xT[kt * 128:(kt + 1) * 128,
                               b * S + t0: b * S + t0 + 1024])
                    xts.append(xt)
                for c2 in range(2):
                    ps = [psB_pool.tile([128, 512], F32, name=f"psB{i}",
                                        tag=f"psB{i}") for i in range(3)]
                    for kt in range(KT_D):
                        for pi, wname in enumerate(("wq", "wk", "wv")):
                            nc.tensor.matmul(
                                ps[pi][:],
                                w_sb[wname][:, kt, :],
                                xts[kt][:, c2 * 512:(c2 + 1) * 512],
                                start=(kt == 0), stop=(kt == KT_D - 1),
                            )
                    for pi, (dname, bname) in enumerate(
                            (("qT", "bq"), ("kT", "bk"), ("vT", "bv"))):
                        nc.vector.tensor_scalar_add(
                            out=qkvT[dname][b][:, t0 + c2 * 512:
                                               t0 + (c2 + 1) * 512],
                            in0=ps[pi][:],
                            scalar1=b_sb[bname][:, 0:1],
                        )

            # ---------------- stage C: V^T -> V_aug for one batch ----------------
            def stage_C(psT_pool, b):
                for h in range(2):
                    pr = b * 2 + h
                    for g in range(2):  # groups of 8 ktiles
                        pst = psT_pool.tile([128, 512], BF16, name="pst", tag="pst")
                        for j in range(8):
                            kt = g * 8 + j
                            nc.tensor.transpose(
                                out=pst[:, j * DH:(j + 1) * DH],
                                in_=qkvT["vT"][b][h * DH:(h + 1) * DH,
                                                  kt * 128:(kt + 1) * 128],
                                identity=ident[h * DH:(h + 1) * DH,
                                               h * DH:(h + 1) * DH],
                            )
                        nc.vector.tensor_copy(
                            vaug[:, pr, g * 8:(g + 1) * 8, 0:DH],
                            pst[:],
                        )

            # ------- stage D/E: one (batch, q-chunk): attention + normalize -------
            def stage_D(pools, b, qc):
                psS_pool, psC_pool, misc_pool, exp_pool, rpool, cn_pool = pools
                pr0, pr1 = b * 2, b * 2 + 1
                qT0 = qkvT["qT"][b][0:DH, :]
                kT0 = qkvT["kT"][b][0:DH, :]
                qT1 = qkvT["qT"][b][DH:2 * DH, :]
                kT1 = qkvT["kT"][b][DH:2 * DH, :]
                q0 = qc * 512
                n_kt = q0 // 128 + 4
                ps_c0 = psC_pool.tile([128, 512], F32, name="ps_c0", tag="ps_ctx")
                ps_c1 = psC_pool.tile([128, 512], F32, name="ps_c1", tag="ps_ctx")
                for kt in range(n_kt):
                    off = max(0, kt * 128 - q0)
                    ps_s = psS_pool.tile([128, 1024], F32, name="ps_s", tag="ps_s")
                    # h0 on PE rows 0-63, h1 on rows 64-127: different row
                    # groups -> the two matmuls run concurrently
                    nc.tensor.matmul(
                        ps_s[:, off:512],
                        kT0[:, kt * 128:(kt + 1) * 128],
                        qT0[:, q0 + off:q0 + 512],
                        start=True, stop=True,
                    )
                    nc.tensor.matmul(
                        ps_s[:, 512 + off:1024],
                        kT1[:, kt * 128:(kt + 1) * 128],
                        qT1[:, q0 + off:q0 + 512],
                        start=True, stop=True,
                    )
                    ex = exp_pool.tile([128, 1024], BF16, name="ex", tag="ex")
                    # one exp over both heads' halves; the gap [512:512+off)
                    # holds stale-but-finite data the ctx matmuls never read
                    nc.scalar.activation(
                        out=ex[:, off:1024], in_=ps_s[:, off:1024],
                        func=mybir.ActivationFunctionType.Exp,
                        scale=INV_SCALE,
                    )
                    if kt * 128 >= q0:
                        # diagonal tile: multiplicative causal mask, applied
                        # AFTER exp so DVE stays off the PE->ACT critical path
                        nc.vector.tensor_mul(
                            out=ex[:, off:off + 128],
                            in0=ex[:, off:off + 128],
                            in1=tri01[:],
                        )
                        nc.vector.tensor_mul(
                            out=ex[:, 512 + off:512 + off + 128],
                            in0=ex[:, 512 + off:512 + off + 128],
                            in1=tri01[:],
                        )
                    nc.tensor.matmul(
                        ps_c0[0:DH + 1, off:512],
                        vaug[:, pr0, kt, :],
                        ex[:, off:512],
                        start=(kt == 0), stop=(kt == n_kt - 1),
                        skip_group_check=True,
                    )
                    nc.tensor.matmul(
                        ps_c1[0:DH + 1, off:512],
                        vaug[:, pr1, kt, :],
                        ex[:, 512 + off:1024],
                        start=(kt == 0), stop=(kt == n_kt - 1),
                        skip_group_check=True,
                    )
                nc.vector.tensor_copy(
                    ctxu[0:DH + 1, pr0, q0:q0 + 512], ps_c0[0:DH + 1, :])
                nc.vector.tensor_copy(
                    ctxu[0:DH + 1, pr1, q0:q0 + 512], ps_c1[0:DH + 1, :])
                # ---- stage E: r = 1/l = exp(-ln(l)), both heads (on ACT: a
                # DVE reciprocal here stalls the DVE FIFO ~5us per q-chunk) ----
                ln_f = rpool.tile([65, 2, 512], F32, name="ln_f", tag="ln_f")
                nc.scalar.activation(
                    out=ln_f[64:65, :, :],
                    in_=ctxu[64:65, pr0:pr0 + 2, q0:q0 + 512],
                    func=mybir.ActivationFunctionType.Ln)
                r_t = rpool.tile([65, 2, 512], F32R, name="r_t", tag="r_t")
                nc.scalar.activation(
                    out=r_t[64:65, :, :], in_=ln_f[64:65, :, :],
                    func=mybir.ActivationFunctionType.Exp, scale=-1.0)
                # normalize ctx^T to bf16, stage into the A2A send buffer
                # blocks for dst ranks 4*(qc%2)..+3 of half qc//2
                cn = cn_pool.tile([128, 512], BF16, name="cn", tag="cn")
                for h in range(2):
                    bcst = misc_pool.tile([128, 512], F32, name="bc", tag="efps")
                    nc.tensor.matmul(
                        bcst[0:DH, :],
                        ones_r[64:65, 0:DH],
                        r_t[64:65, h, :],
                        start=True, stop=True,
                    )
                    nc.vector.tensor_mul(
                        out=cn[h * DH:(h + 1) * DH, :],
                        in0=ctxu[0:DH, b * 2 + h, q0:q0 + 512],
                        in1=bcst[0:DH, :],
                    )
                hf, qq = qc // 2, qc % 2
                for j in range(4):
                    nc.sync.dma_start(
                        out=a2i[b][hf][4 * qq + j, :, :],
                        in_=cn[:, j * HT:(j + 1) * HT])
                if do_collective and qq == 1:
                    nc.gpsimd.collective_compute(
                        "AllToAll",
                        mybir.AluOpType.bypass,
                        ins=[a2i[b][hf][:]],
                        outs=[a2o[b][hf][:]],
                        replica_groups=[list(range(NC))],
                    )

            def gx_load(gx_pool, b, hf):
                gx = gx_pool.tile([128, KT_D, HT], BF16, name="gx", tag="gx")
                nc.sync.dma_start(
                    out=gx[:],
                    in_=a2o[b][hf][:].rearrange("kt p t -> p kt t"))
                return gx

            def stage_F(misc_pool, yo_pool, b, hf, gx):
                # out^T[all od, my 128 tokens of half hf] from resharded ctx^T
                tcol = (2 * b + hf) * HT
                for ot in range(KT_D):
                    ps_o = misc_pool.tile([128, 512], F32, name="ps_o", tag="efps")
                    for kt in range(KT_D):
                        nc.tensor.matmul(
                            ps_o[:, 0:HT],
                            wo_sb[:, kt, ot * 128:(ot + 1) * 128],
                            gx[:, kt, :],
                            start=(kt == 0), stop=(kt == KT_D - 1),
                        )
                    yo = yo_pool.tile([128, HT], F32, name="yo", tag="yo")
                    nc.vector.tensor_scalar_add(
                        out=yo[:], in0=ps_o[:, 0:HT],
                        scalar1=bo_sb[:, ot:ot + 1])
                    nc.sync.dma_start(
                        out=yT[ot * 128:(ot + 1) * 128, tcol:tcol + HT],
                        in_=yo[:])

            def b_scope():
                return (
                    tc.tile_pool(name="xt_pool", bufs=10),
                    tc.tile_pool(name="psB", bufs=2, space="PSUM"),
                    tc.tile_pool(name="psT", bufs=2, space="PSUM"),
                )

            def d_scope():
                return (
                    tc.tile_pool(name="psS", bufs=2, space="PSUM"),
                    tc.tile_pool(name="psC", bufs=2, space="PSUM"),
                    tc.tile_pool(name="misc", bufs=2, space="PSUM"),
                    tc.tile_pool(name="exp_pool", bufs=4),
                    tc.tile_pool(name="rpool", bufs=2),
                    tc.tile_pool(name="cn_pool", bufs=3),
                )

            # ---- segment 1: B(b0) + C(b0) ----
            xp, bp, tp = b_scope()
            with xp as xt_pool, bp as psB_pool, tp as psT_pool:
                stage_B(xt_pool, psB_pool, 0, 0)
                stage_B(xt_pool, psB_pool, 0, 1)
                stage_C(psT_pool, 0)
            # ---- segment 2: D(b0, qc0-1); wo loads in background ----
            nc.gpsimd.dma_start(
                out=wo_sb[:], in_=wo[:].rearrange("p (kt n) -> p kt n", kt=KT_D))
            s, c, m, e, r, cnp = d_scope()
            with s as s_, c as c_, m as m_, e as e_, r as r_, cnp as cn_:
                pools = (s_, c_, m_, e_, r_, cn_)
                stage_D(pools, 0, 0)
                stage_D(pools, 0, 1)   # issues A2A(b0, half0)
            # ---- segment 3: B(b1, tq0) — PE work under D(b0)'s ACT tail ----
            xp, bp, tp = b_scope()
            with xp as xt_pool, bp as psB_pool:
                stage_B(xt_pool, psB_pool, 1, 0)
            # ---- segment 4: D(b0, qc2-3) ----
            s, c, m, e, r, cnp = d_scope()
            with s as s_, c as c_, m as m_, e as e_, r as r_, cnp as cn_:
                pools = (s_, c_, m_, e_, r_, cn_)
                stage_D(pools, 0, 2)
                stage_D(pools, 0, 3)   # issues A2A(b0, half1)
            # ---- segment 5: B(b1, tq1) + C(b1) ----
            xp, bp, tp = b_scope()
            with xp as xt_pool, bp as psB_pool, tp as psT_pool:
                stage_B(xt_pool, psB_pool, 1, 1)
                stage_C(psT_pool, 1)
            # ---- segment 6: D(b1) with F(b0) interleaved, then F(b1) ----
            s, c, m, e, r, cnp = d_scope()
            with (s as s_, c as c_, m as m_, e as e_, r as r_, cnp as cn_,
                  tc.tile_pool(name="gx_pool", bufs=4) as gx_pool,
                  tc.tile_pool(name="yo_pool", bufs=2) as yo_pool):
                pools = (s_, c_, m_, e_, r_, cn_)
                gx00 = gx_load(gx_pool, 0, 0)
                stage_D(pools, 1, 0)
                stage_D(pools, 1, 1)   # issues A2A(b1, half0)
                gx01 = gx_load(gx_pool, 0, 1)
                stage_F(m_, yo_pool, 0, 0, gx00)
                stage_D(pools, 1, 2)
                stage_F(m_, yo_pool, 0, 1, gx01)
                stage_D(pools, 1, 3)   # issues A2A(b1, half1)
                gx10 = gx_load(gx_pool, 1, 0)
                stage_F(m_, yo_pool, 1, 0, gx10)
                gx11 = gx_load(gx_pool, 1, 1)
                stage_F(m_, yo_pool, 1, 1, gx11)

    _split_waits(nc)
    return nc


def _swz(w):
    """[D, n] -> preswizzled [128, KT_D*n]: row p = concat_kt W[kt*128+p, :]."""
    n = w.shape[1]
    return np.ascontiguousarray(
        w.reshape(KT_D, 128, n).transpose(1, 0, 2).reshape(128, KT_D * n)
        .astype(BFNP))


def kernel(x, mask, Wq, bq, Wk, bk, Wv, bv, Wo, bo, trace=False, repeat=1, _in_maps_only=False):
    x = np.asarray(x, dtype=np.float32).reshape(T, D)
    xT = np.ascontiguousarray(x.T.astype(BFNP))
    wo_full = _swz(np.asarray(Wo, np.float32))
    bo_full = np.ascontiguousarray(
        np.asarray(bo, np.float32).reshape(KT_D, 128).T)
    in_maps = []
    for c in range(NC):
        sl = slice(c * HG, (c + 1) * HG)
        in_maps.append({
            "xT": xT,
            "wq": _swz(np.asarray(Wq, np.float32)[:, sl]),
            "wk": _swz(np.asarray(Wk, np.float32)[:, sl]),
            "wv": _swz(np.asarray(Wv, np.float32)[:, sl]),
            "wo": wo_full,
            "bq": np.ascontiguousarray(np.asarray(bq, np.float32)[sl].reshape(HG, 1)),
            "bk": np.ascontiguousarray(np.asarray(bk, np.float32)[sl].reshape(HG, 1)),
            "bv": np.ascontiguousarray(np.asarray(bv, np.float32)[sl].reshape(HG, 1)),
            "bo": bo_full,
        })
    if _in_maps_only:
        return in_maps
    nc = build_module()
    res = run_bass_kernel_spmd(nc, in_maps, core_ids=list(range(NC)), trace=trace)
    out = np.empty((B, S, D), dtype=np.float32)
    for c in range(NC):
        yt = res.results[c]["yT"]  # [D, (2b+hf)*128 columns]
        for b in range(B):
            for hf in range(2):
                t0 = hf * 1024 + c * 128
                out[b, t0:t0 + 128, :] = yt[:, (2 * b + hf) * 128:
                                            (2 * b + hf + 1) * 128].T
    if trace:
        kernel.last_results = res
    return out


# revision 27
# speedup vs baseline: 1.0636x; 1.0636x over previous
"""Multi-head causal self-attention (B=2, S=2048, D=1024, H=16) on 8 TRN2 cores.

Sharding: head-parallel attention + token-parallel output projection.
Core c owns head-group c = heads {2c, 2c+1} (= 128 of the 1024 qkv dims,
both batches) for stages B-E, then tokens [256c, 256c+256) of each batch
for stage F (Wo replicated).

Per core:
  stage B: Q^T/K^T/V^T = (x @ W{q,k,v}[:, c-slice] + b)^T            [128, 4096]
           (bf16 operands, f32 PSUM accumulation, 512-token PSUM chunks
           double-buffered)
  stage C: V^T -> V_aug [tok, 65] tiles (col 64 = ones, for the l-row trick)
  stage D: per (batch, q-chunk, k-tile): scores^T for BOTH heads with one
           [128,1024] PSUM tile — h0 on PE rows 0-63, h1 on rows 64-127
           (different row groups -> the two matmuls run concurrently).
           One exp (ACT, scale=1/8, bf16) covers both heads; causal mask
           (DVE) on diagonal tiles; ctx^T accumulation per head with the
           ones column producing l = sum(exp) in row 64.
  stage E: per (batch, q-chunk): r = 1/l (Ln+Exp), broadcast via PE outer
           product, normalize ctx^T to bf16, DMA into the AllToAll send
           buffer laid out as [dst-rank, 128 dims, 256 tokens].
           Per batch: AllToAll reshards ctx^T so each core holds ALL 1024
           ctx dims for its 256 tokens (~0.5 MB wire per rank — 8x less
           than gathering full ctx on every core).
  stage F: out^T[all od, my tokens] = Wo^T-tiles @ resharded ctx^T + bo.
           F(b0) runs while batch 1's AllToAll is in flight.

Host: passes x pre-transposed in bf16, wq/wk/wv column slices, FULL Wo in
bf16; reassembles from per-core token slices.
"""

import sys

for p in ("/opt/trn_rl_repo", "/root/.axon_site/_ro/trn_rl_repo"):
    if p not in sys.path:
        sys.path.insert(0, p)

import numpy as np
import ml_dtypes

import bass_rust
import concourse.bass as bass
import concourse.mybir as mybir
from concourse.bass_utils import run_bass_kernel_spmd
from concourse.masks import make_identity
from concourse.tile import TileContext

B, S, D = 2, 2048, 1024
H, DH = 16, 64
T = B * S              # 4096 tokens
NC = 8                 # cores
HG = D // NC           # 128 qkv dims per core (2 heads)
TC = S // NC           # 256 tokens per core per batch (stage F)
KT_D = D // 128        # 8 contraction tiles over d_model
NQC = S // 512         # 4 q-chunks per batch
INV_SCALE = 1.0 / float(np.sqrt(DH))  # 1/8
F32 = mybir.dt.float32
F32R = mybir.dt.float32r
BF16 = mybir.dt.bfloat16
BFNP = ml_dtypes.bfloat16


def _split_waits(nc, max_waits=1):
    """This walrus build accepts one sync-wait per instruction; Tile sometimes
    emits more. Split extras into preceding NoOps on the same engine."""
    n = 0
    for f in nc.m.functions:
        for bb in f.blocks:
            out = []
            for inst in bb.instructions:
                si = getattr(inst, "sync_info", None)
                if si is not None and si.on_wait and len(si.on_wait) > max_waits:
                    waits = list(si.on_wait)
                    head, rest = waits[:-max_waits], waits[-max_waits:]
                    k = 0
                    while head:
                        chunk, head = head[:max_waits], head[max_waits:]
                        out.append(mybir.InstNoOp(
                            name=f"{inst.name}-wsplit-{k}", ins=[], outs=[],
                            engine=inst.engine,
                            sync_info=bass_rust.SyncInfo(on_wait=chunk, on_update=[]),
                        ))
                        k += 1
                    si.on_wait = rest
                    n += 1
                out.append(inst)
            bb.instructions = out
    return n


def build_module(repeat=1, stages="BCDEF", do_collective=True):
    nc = bass.Bass()

    xT = nc.dram_tensor("xT", [D, T], BF16, kind="ExternalInput")
    # weights arrive host-preswizzled as [128, kt*n]: row p holds
    # W[kt*128+p, n] for kt-major n-minor — one contiguous DMA line per
    # partition instead of 1024 small descriptors
    wq = nc.dram_tensor("wq", [128, KT_D * HG], BF16, kind="ExternalInput")
    wk = nc.dram_tensor("wk", [128, KT_D * HG], BF16, kind="ExternalInput")
    wv = nc.dram_tensor("wv", [128, KT_D * HG], BF16, kind="ExternalInput")
    wo = nc.dram_tensor("wo", [128, KT_D * D], BF16, kind="ExternalInput")
    bq = nc.dram_tensor("bq", [HG, 1], F32, kind="ExternalInput")
    bk = nc.dram_tensor("bk", [HG, 1], F32, kind="ExternalInput")
    bv = nc.dram_tensor("bv", [HG, 1], F32, kind="ExternalInput")
    bo = nc.dram_tensor("bo", [128, KT_D], F32, kind="ExternalInput")
    yT = nc.dram_tensor("yT", [D, B * 2 * 128], F32, kind="ExternalOutput")

    # AllToAll staging per (batch, half): send block j = (my 128 dims,
    # 128 tokens of rank j); receive block j = (rank j's 128 dims, my
    # 128 tokens of that half)
    HT = 128  # tokens per rank per half-batch
    a2i = [[nc.dram_tensor(f"a2i{b}_{h}", [NC, HG, HT], BF16)
            for h in range(2)] for b in range(B)]
    a2o = [[nc.dram_tensor(f"a2o{b}_{h}", [NC, HG, HT], BF16)
            for h in range(2)] for b in range(B)]

    with TileContext(nc) as tc:
        with tc.tile_pool(name="persist", bufs=1) as pp:
            # qkv weights as [128, kt, 128]; full Wo as [128, kt, 1024]
            # (contraction chunk is the partition dim). Spread the loads
            # across engine queues so the first x tiles aren't stuck
            # behind the 2MB Wo load on the sync queue.
            w_sb = {}
            for name, dram, eng in (("wq", wq, nc.sync), ("wk", wk, nc.scalar),
                                    ("wv", wv, nc.scalar)):
                t = pp.tile([128, KT_D, HG], BF16, name=f"{name}_sb", tag=f"{name}_sb")
                eng.dma_start(out=t[:], in_=dram[:].rearrange("p (kt n) -> p kt n", kt=KT_D))
                w_sb[name] = t
            # wo_sb tile allocated here; its 2MB DMA is emitted after stage B
            # so it doesn't steal DMA bandwidth from the startup x/w loads
            wo_sb = pp.tile([128, KT_D, D], BF16, name="wo_sb", tag="wo_sb")
            b_sb = {}
            for name, dram in (("bq", bq), ("bk", bk), ("bv", bv)):
                t = pp.tile([HG, 1], F32, name=f"{name}_sb", tag=f"{name}_sb")
                nc.sync.dma_start(out=t[:], in_=dram[:])
                b_sb[name] = t
            bo_sb = pp.tile([128, KT_D], F32, name="bo_sb", tag="bo_sb")
            nc.sync.dma_start(out=bo_sb[:], in_=bo[:])

            # identity built in f32 (gpsimd memset can't write bf16 reliably),
            # then DVE-copied (rounds) into the bf16 tile matmul needs
            ident_f = pp.tile([128, 128], F32, name="ident_f", tag="ident_f")
            make_identity(nc, ident_f[:])
            ident = pp.tile([128, 128], BF16, name="ident", tag="ident")
            nc.vector.tensor_copy(ident[:], ident_f[:])
            # multiplicative causal mask for a diagonal 128x128 tile in
            # scores^T: tri01[r, c] = 1 where r <= c (k <= q), else 0
            tri_f = pp.tile([128, 128], F32, name="tri_f", tag="tri_f")
            nc.gpsimd.memset(tri_f[:], 1.0)
            nc.gpsimd.affine_select(
                out=tri_f[:], in_=tri_f[:],
                compare_op=mybir.AluOpType.is_ge, fill=0.0,
                base=0, pattern=[[1, 128]], channel_multiplier=-1,
            )
            tri01 = pp.tile([128, 128], BF16, name="tri01", tag="tri01")
            nc.vector.tensor_copy(tri01[:], tri_f[:])
            # ones row for the r-broadcast outer product (f32r, full speed)
            ones_r = pp.tile([65, 128], F32R, name="ones_r", tag="ones_r")
            of = pp.tile([65, 128], F32, name="of", tag="of")
            nc.vector.memset(of[:], 1.0)
            nc.vector.tensor_copy(ones_r[:], of[:])
            ones128 = pp.tile([128, 64], F32, name="ones128", tag="ones128")
            nc.vector.memset(ones128[:], 1.0)

            # per-batch Q^T/K^T/V^T so batch 1's projection overlaps batch 0's
            # attention
            qkvT = {}
            for name in ("qT", "kT", "vT"):
                qkvT[name] = [pp.tile([128, S], BF16, name=f"{name}{b}", tag=f"{name}{b}")
                              for b in range(B)]

            vaug = pp.tile([128, B * 2, S // 128, DH + 1], BF16, name="vaug", tag="vaug")
            nc.vector.tensor_copy(vaug[:, :, :, DH:DH + 1], ones128[:, :])
            # [65 used partitions, pair, q]; row 64 = l
            ctxu = pp.tile([128, B * 2, S], F32, name="ctxu", tag="ctxu")

            for _rep in range(repeat):
                if "B" not in stages:
                    break
                # ---------------- stage B: QKV projections (both batches) ----------------
                with (
                    tc.tile_pool(name="xt_pool", bufs=12) as xt_pool,
                    tc.tile_pool(name="psB", bufs=2, space="PSUM") as psB_pool,
                    tc.tile_pool(name="psT", bufs=2, space="PSUM") as psT_pool,
                ):
                    for b in range(B):
                        for tq in range(2):
                            t0 = tq * 1024
                            xts = []
                            for kt in range(KT_D):
                                xt = xt_pool.tile([128, 1024], BF16, name="xt", tag="xt")
                                # the very first chunk spreads its loads over
                                # the idle gpsimd/scalar/vector queues — early
                                # DMA throughput is per-queue limited and these
                                # bytes gate the first matmul (wq occupies sync)
                                if (b, tq) != (0, 0):
                                    eng = nc.sync
                                elif kt % 3 == 0:
                                    eng = nc.gpsimd
                                else:
                                    eng = nc.scalar if kt % 3 == 1 else nc.sync
                                eng.dma_start(
                                    out=xt[:],
                                    in_=xT[kt * 128:(kt + 1) * 128,
                                           b * S + t0: b * S + t0 + 1024])
                                xts.append(xt)
                            for c2 in range(2):
                                ps = [psB_pool.tile([128, 512], F32, name=f"psB{i}",
                                                    tag=f"psB{i}") for i in range(3)]
                                for kt in range(KT_D):
                                    for pi, wname in enumerate(("wq", "wk", "wv")):
                                        nc.tensor.matmul(
                                            ps[pi][:],
                                            w_sb[wname][:, kt, :],
                                            xts[kt][:, c2 * 512:(c2 + 1) * 512],
                                            start=(kt == 0), stop=(kt == KT_D - 1),
                                        )
                                for pi, (dname, bname) in enumerate(
                                        (("qT", "bq"), ("kT", "bk"), ("vT", "bv"))):
                                    nc.vector.tensor_scalar_add(
                                        out=qkvT[dname][b][:, t0 + c2 * 512:
                                                           t0 + (c2 + 1) * 512],
                                        in0=ps[pi][:],
                                        scalar1=b_sb[bname][:, 0:1],
                                    )
                        if "C" not in stages:
                            continue
                        # ---- stage C: V^T -> V_aug for batch b ----
                        for h in range(2):
                            pr = b * 2 + h
                            for g in range(2):  # groups of 8 ktiles
                                pst = psT_pool.tile([128, 512], BF16, name="pst", tag="pst")
                                for j in range(8):
                                    kt = g * 8 + j
                                    nc.tensor.transpose(
                                        out=pst[:, j * DH:(j + 1) * DH],
                                        in_=qkvT["vT"][b][h * DH:(h + 1) * DH,
                                                          kt * 128:(kt + 1) * 128],
                                        identity=ident[h * DH:(h + 1) * DH,
                                                       h * DH:(h + 1) * DH],
                                    )
                                nc.vector.tensor_copy(
                                    vaug[:, pr, g * 8:(g + 1) * 8, 0:DH],
                                    pst[:],
                                )
                if "D" not in stages:
                    continue
                # wo load now — overlaps stage D, needed only by stage F
                nc.gpsimd.dma_start(
                    out=wo_sb[:], in_=wo[:].rearrange("p (kt n) -> p kt n", kt=KT_D))
                # ------- stages D-F, pipelined per (batch, q-chunk) -------
                with (
                    tc.tile_pool(name="psS", bufs=2, space="PSUM") as psS_pool,
                    tc.tile_pool(name="psC", bufs=2, space="PSUM") as psC_pool,
                    tc.tile_pool(name="misc", bufs=2, space="PSUM") as misc_pool,
                    tc.tile_pool(name="exp_pool", bufs=4) as exp_pool,
                    tc.tile_pool(name="rpool", bufs=2) as rpool,
                    tc.tile_pool(name="cn_pool", bufs=3) as cn_pool,
                    tc.tile_pool(name="gx_pool", bufs=4) as gx_pool,
                    tc.tile_pool(name="yo_pool", bufs=2) as yo_pool,
                ):
                    def stage_D(b):
                        pr0, pr1 = b * 2, b * 2 + 1
                        qT0 = qkvT["qT"][b][0:DH, :]
                        kT0 = qkvT["kT"][b][0:DH, :]
                        qT1 = qkvT["qT"][b][DH:2 * DH, :]
                        kT1 = qkvT["kT"][b][DH:2 * DH, :]
                        for qc in range(NQC):
                            q0 = qc * 512
                            n_kt = q0 // 128 + 4
                            # ---- stage D: both heads interleaved per k-tile ----
                            ps_c0 = psC_pool.tile([128, 512], F32, name="ps_c0",
                                                  tag="ps_ctx")
                            ps_c1 = psC_pool.tile([128, 512], F32, name="ps_c1",
                                                  tag="ps_ctx")
                            for kt in range(n_kt):
                                off = max(0, kt * 128 - q0)
                                ps_s = psS_pool.tile([128, 1024], F32, name="ps_s",
                                                     tag="ps_s")
                                # h0 on PE rows 0-63, h1 on rows 64-127: the two
                                # matmuls occupy different row groups and run
                                # concurrently
                                nc.tensor.matmul(
                                    ps_s[:, off:512],
                                    kT0[:, kt * 128:(kt + 1) * 128],
                                    qT0[:, q0 + off:q0 + 512],
                                    start=True, stop=True,
                                )
                                nc.tensor.matmul(
                                    ps_s[:, 512 + off:1024],
                                    kT1[:, kt * 128:(kt + 1) * 128],
                                    qT1[:, q0 + off:q0 + 512],
                                    start=True, stop=True,
                                )
                                ex = exp_pool.tile([128, 1024], BF16, name="ex", tag="ex")
                                # one exp over both heads' halves; the gap
                                # [512:512+off) holds stale-but-finite data
                                # that the ctx matmuls never read.
                                nc.scalar.activation(
                                    out=ex[:, off:1024], in_=ps_s[:, off:1024],
                                    func=mybir.ActivationFunctionType.Exp,
                                    scale=INV_SCALE,
                                )
                                if kt * 128 >= q0:
                                    # diagonal tile: multiplicative causal mask,
                                    # applied AFTER exp so DVE stays off the
                                    # PE->ACT critical path
                                    nc.vector.tensor_mul(
                                        out=ex[:, off:off + 128],
                                        in0=ex[:, off:off + 128],
                                        in1=tri01[:],
                                    )
                                    nc.vector.tensor_mul(
                                        out=ex[:, 512 + off:512 + off + 128],
                                        in0=ex[:, 512 + off:512 + off + 128],
                                        in1=tri01[:],
                                    )
                                nc.tensor.matmul(
                                    ps_c0[0:DH + 1, off:512],
                                    vaug[:, pr0, kt, :],
                                    ex[:, off:512],
                                    start=(kt == 0), stop=(kt == n_kt - 1),
                                    skip_group_check=True,
                                )
                                nc.tensor.matmul(
                                    ps_c1[0:DH + 1, off:512],
                                    vaug[:, pr1, kt, :],
                                    ex[:, 512 + off:1024],
                                    start=(kt == 0), stop=(kt == n_kt - 1),
                                    skip_group_check=True,
                                )
                            nc.vector.tensor_copy(
                                ctxu[0:DH + 1, pr0, q0:q0 + 512], ps_c0[0:DH + 1, :])
                            nc.vector.tensor_copy(
                                ctxu[0:DH + 1, pr1, q0:q0 + 512], ps_c1[0:DH + 1, :])
                            if "E" not in stages:
                                continue
                            # ---- stage E: r = 1/l = exp(-ln(l)), both heads
                            # (on ACT: a DVE reciprocal here stalls the DVE
                            # FIFO and costs ~5us of PE time per q-chunk) ----
                            ln_f = rpool.tile([65, 2, 512], F32, name="ln_f", tag="ln_f")
                            nc.scalar.activation(
                                out=ln_f[64:65, :, :],
                                in_=ctxu[64:65, pr0:pr0 + 2, q0:q0 + 512],
                                func=mybir.ActivationFunctionType.Ln)
                            r_t = rpool.tile([65, 2, 512], F32R, name="r_t", tag="r_t")
                            nc.scalar.activation(
                                out=r_t[64:65, :, :], in_=ln_f[64:65, :, :],
                                func=mybir.ActivationFunctionType.Exp, scale=-1.0)
                            # normalize ctx^T to bf16, stage into the A2A send
                            # buffer blocks for dst ranks 4*(qc%2)..+3 of half qc//2
                            cn = cn_pool.tile([128, 512], BF16, name="cn", tag="cn")
                            for h in range(2):
                                bcst = misc_pool.tile([128, 512], F32, name="bc",
                                                      tag="efps")
                                nc.tensor.matmul(
                                    bcst[0:DH, :],
                                    ones_r[64:65, 0:DH],
                                    r_t[64:65, h, :],
                                    start=True, stop=True,
                                )
                                nc.vector.tensor_mul(
                                    out=cn[h * DH:(h + 1) * DH, :],
                                    in0=ctxu[0:DH, b * 2 + h, q0:q0 + 512],
                                    in1=bcst[0:DH, :],
                                )
                            hf, qq = qc // 2, qc % 2
                            for j in range(4):
                                nc.sync.dma_start(
                                    out=a2i[b][hf][4 * qq + j, :, :],
                                    in_=cn[:, j * HT:(j + 1) * HT])
                            if do_collective and qq == 1:
                                nc.gpsimd.collective_compute(
                                    "AllToAll",
                                    mybir.AluOpType.bypass,
                                    ins=[a2i[b][hf][:]],
                                    outs=[a2o[b][hf][:]],
                                    replica_groups=[list(range(NC))],
                                )

                    def gx_load(b, hf, eng=None):
                        gx = gx_pool.tile([128, KT_D, HT], BF16, name="gx", tag="gx")
                        (eng or nc.sync).dma_start(
                            out=gx[:],
                            in_=a2o[b][hf][:].rearrange("kt p t -> p kt t"))
                        return gx

                    def stage_F(b, hf, gx):
                        # out^T[all od, my 128 tokens of half hf] from
                        # resharded ctx^T
                        tcol = (2 * b + hf) * HT
                        for ot in range(KT_D):
                            ps_o = misc_pool.tile([128, 512], F32, name="ps_o",
                                                  tag="efps")
                            for kt in range(KT_D):
                                nc.tensor.matmul(
                                    ps_o[:, 0:HT],
                                    wo_sb[:, kt, ot * 128:(ot + 1) * 128],
                                    gx[:, kt, :],
                                    start=(kt == 0), stop=(kt == KT_D - 1),
                                )
                            yo = yo_pool.tile([128, HT], F32, name="yo", tag="yo")
                            nc.vector.tensor_scalar_add(
                                out=yo[:], in0=ps_o[:, 0:HT],
                                scalar1=bo_sb[:, ot:ot + 1])
                            nc.sync.dma_start(
                                out=yT[ot * 128:(ot + 1) * 128, tcol:tcol + HT],
                                in_=yo[:])

                    stage_D(0)
                    if "F" in stages and "E" in stages and do_collective:
                        # prefetch batch 0's resharded halves during batch 1's
                        # attention; batch 0's F fills batch 1's A2A latency
                        gx00 = gx_load(0, 0)
                        gx01 = gx_load(0, 1)
                        stage_D(1)
                        # spread the tail reshard loads over idle queues so
                        # the four 256B-line gathers run on parallel DMA
                        # engines instead of serializing on the sync queue
                        gx10 = gx_load(1, 0, nc.scalar)
                        gx11 = gx_load(1, 1, nc.gpsimd)
                        stage_F(0, 0, gx00)
                        stage_F(0, 1, gx01)
                        stage_F(1, 0, gx10)
                        stage_F(1, 1, gx11)
                    else:
                        stage_D(1)

    _split_waits(nc)
    return nc


def _swz(w):
    """[D, n] -> preswizzled [128, KT_D*n]: row p = concat_kt W[kt*128+p, :]."""
    n = w.shape[1]
    return np.ascontiguousarray(
        w.reshape(KT_D, 128, n).transpose(1, 0, 2).reshape(128, KT_D * n)
        .astype(BFNP))


def kernel(x, mask, Wq, bq, Wk, bk, Wv, bv, Wo, bo, trace=False, repeat=1, _in_maps_only=False):
    x = np.asarray(x, dtype=np.float32).reshape(T, D)
    xT = np.ascontiguousarray(x.T.astype(BFNP))
    wo_full = _swz(np.asarray(Wo, np.float32))
    bo_full = np.ascontiguousarray(
        np.asarray(bo, np.float32).reshape(KT_D, 128).T)
    in_maps = []
    for c in range(NC):
        sl = slice(c * HG, (c + 1) * HG)
        in_maps.append({
            "xT": xT,
            "wq": _swz(np.asarray(Wq, np.float32)[:, sl]),
            "wk": _swz(np.asarray(Wk, np.float32)[:, sl]),
            "wv": _swz(np.asarray(Wv, np.float32)[:, sl]),
            "wo": wo_full,
            "bq": np.ascontiguousarray(np.asarray(bq, np.float32)[sl].reshape(HG, 1)),
            "bk": np.ascontiguousarray(np.asarray(bk, np.float32)[sl].reshape(HG, 1)),
            "bv": np.ascontiguousarray(np.asarray(bv, np.float32)[sl].reshape(HG, 1)),
            "bo": bo_full,
        })
    if _in_maps_only:
        return in_maps
    nc = build_module(repeat=repeat)
    res = run_bass_kernel_spmd(nc, in_maps, core_ids=list(range(NC)), trace=trace)
    out = np.empty((B, S, D), dtype=np.float32)
    for c in range(NC):
        yt = res.results[c]["yT"]  # [D, (2b+hf)*128 columns]
        for b in range(B):
            for hf in range(2):
                t0 = hf * 1024 + c * 128
                out[b, t0:t0 + 128, :] = yt[:, (2 * b + hf) * 128:
                                            (2 * b + hf + 1) * 128].T
    if trace:
        kernel.last_results = res
    return out
